# revision 1
# baseline (speedup 1.0000x reference)
"""Trainium2 Bass kernel for AttentiveReduce (segment-softmax attention readout).

reference semantics (uniform segments of S=64 nodes):
    score = leakyrelu(feat @ w, 0.2)            # (N,)
    alpha = segment_softmax(score)               # softmax within each segment
    out[g, :] = sum_{n in seg g} alpha[n] * feat[n, :]   # (B, D)

Sharding: 8 cores, core c owns segments [c*2048, (c+1)*2048) == rows
[c*131072, (c+1)*131072) of feat.  One SPMD Bass program; no collectives.

Per-core layout (node-major):
  macro-tile = 4096 nodes = 32 tiles of 128 nodes = 64 segments (2 MiB fp32).
  ft[p, t, d] = feat[base + t*128 + p, d]   (partition = node-in-tile)

Engines:
  phase1 (per macro): DMA load; score multiply (DVE or GPSIMD, balanced);
    segmented reduce + leakyrelu (DVE).
  phase2 (per macro): PE transpose of the tiny score block; exp straight
    from PSUM on ACT (scores are O(3), so the softmax max-shift is dropped —
    mathematically identical, fp32-safe); denominators + alpha (DVE);
    alpha back via PE; masked pair matrix; weighted segment sums as 32
    TensorE matmuls into a transposed (d, seg) PSUM tile; PE transpose
    back; DMA out.
  The two phases are emitted offset by one macro so every engine's static
  instruction order interleaves the next macro's heavy ops with the previous
  macro's latency-bound softmax chain.
"""

import numpy as np

N_FULL = 1048576
B_FULL = 16384
D = 128
P = 128
S = 64                      # nodes per segment (uniform fast path)
NCORES = 8
NODES_C = N_FULL // NCORES  # 131072
SEGS_C = B_FULL // NCORES   # 2048
T = 32                      # 128-node tiles per macro-tile
MACRO_NODES = P * T         # 4096
MACRO_SEGS = 2 * T          # 64
MACROS = NODES_C // MACRO_NODES  # 32
NEG_SLOPE = 0.2

# Macros whose big score-multiply runs on GPSIMD instead of DVE
# (DVE also owns the segmented reduce; GPSIMD TT is ~1.9x slower).
GPSIMD_MULT_MACROS = frozenset(m for m in range(MACROS) if m % 4 != 0)

_PROGRAM = None
TRACE = False
LAST_RESULT = None


def _numpy_fallback(feat, sizes, w):
    """General segment sizes (not expected in practice)."""
    sizes = sizes.astype(np.int64)
    seg_ids = np.repeat(np.arange(len(sizes)), sizes)
    score = (feat.astype(np.float32) @ w.astype(np.float32))[:, 0]
    score = np.where(score >= 0, score, np.float32(NEG_SLOPE) * score)
    B = len(sizes)
    segmax = np.full(B, -np.inf, np.float32)
    np.maximum.at(segmax, seg_ids, score)
    e = np.exp(score - segmax[seg_ids])
    den = np.zeros(B, np.float32)
    np.add.at(den, seg_ids, e)
    a = (e / den[seg_ids])[:, None].astype(np.float32)
    out = np.zeros((B, feat.shape[1]), np.float32)
    np.add.at(out, seg_ids, feat * a)
    return out


def _build_program_stub():
    """Near-empty program: bounds the per-call dispatch/runtime overhead."""
    import concourse.bacc as bacc
    import concourse.tile as tile
    from concourse import mybir

    f32 = mybir.dt.float32
    nc = bacc.Bacc("TRN2", target_bir_lowering=False, debug=False)
    feat = nc.dram_tensor("feat", [NODES_C, D], f32, kind="ExternalInput")
    wb_d = nc.dram_tensor("wb", [P, D], f32, kind="ExternalInput")
    rwb_d = nc.dram_tensor("rwb", [P, D], f32, kind="ExternalInput")
    mask2_d = nc.dram_tensor("mask2", [P, 2], f32, kind="ExternalInput")
    ident_d = nc.dram_tensor("ident", [P, P], f32, kind="ExternalInput")
    out_d = nc.dram_tensor("out", [SEGS_C, D], f32, kind="ExternalOutput")
    with tile.TileContext(nc) as tc:
        with tc.tile_pool(name="p", bufs=1) as pool:
            wb = pool.tile([P, D], f32)
            nc.sync.dma_start(out=wb[:], in_=wb_d[:, :])
            osb = pool.tile([SEGS_C // 16, 16, D], f32)
            nc.vector.tensor_copy(
                osb[:], wb[:][:, None, :].broadcast_to([P, 16, D])
            )
            nc.sync.dma_start(
                out=out_d[:, :].rearrange("(p s) d -> p s d", p=P), in_=osb[:]
            )
    nc.finalize()
    return nc


def _build_program_dma(cast=True, repeat=1, layout="c", featp_bufs=6):
    """DMA-only probe: just the 32 macro feat loads + a token output."""
    import concourse.bacc as bacc
    import concourse.tile as tile
    from concourse import mybir

    TC, J = 8, 4
    MACRO_NODES = P * TC * J
    MACROS = NODES_C // MACRO_NODES
    f32 = mybir.dt.float32
    bf = mybir.dt.bfloat16
    dt = bf if cast else f32

    nc = bacc.Bacc("TRN2", target_bir_lowering=False, debug=False)
    feat = nc.dram_tensor("feat", [NODES_C, D], f32, kind="ExternalInput")
    out_d = nc.dram_tensor("out", [SEGS_C, D], f32, kind="ExternalOutput")
    with tile.TileContext(nc) as tc:
        with (
            tc.tile_pool(name="featp", bufs=featp_bufs) as featp,
            tc.tile_pool(name="op", bufs=1) as op,
        ):
            o = op.tile([P, SEGS_C // P, D], f32)
            nc.vector.memset(o[:], 0.0)
            last = []
            for _ in range(repeat):
                for m in range(MACROS):
                    nb = m * MACRO_NODES
                    ft = featp.tile([P, TC, J, D], dt)
                    if layout == "c":
                        src = feat[nb:nb + MACRO_NODES, :].rearrange(
                            "(t p j) d -> p t j d", p=P, j=J
                        )
                    else:  # original 512B-chunk layout
                        src = feat[nb:nb + MACRO_NODES, :].rearrange(
                            "(t p) d -> p t d", p=P
                        ).rearrange("p (t j) d -> p t j d", j=J)
                    eng = nc.gpsimd if cast else nc.sync
                    eng.dma_start(out=ft[:], in_=src)
                    last.append(ft)
                    if len(last) > featp_bufs - 1:
                        # consume the tile so the pool can rotate
                        last.pop(0)
            nc.sync.dma_start(
                out=out_d[:, :].rearrange("(p s) d -> p s d", p=P), in_=o[:]
            )
    nc.finalize()
    return nc


def _build_program_c(featp_bufs=6, tmpp_bufs=2, skip_load=False,
                     skip_mm=False, fold_pool=False, repeat=1,
                     n_macros=None, dbg=False, mat_w=False, a8_pool=False,
                     a8_narrow=True, lag=1, short_score=False,
                     psum_dma=False, fold_stop=16):
    """Layout-C kernel: 4 consecutive nodes per SBUF partition.

    node n = macro_base + t*512 + p*4 + j   (t<8 tiles, p<128, j<4)
    so each partition's DMA chunk is 4 rows = 2 KiB contiguous, and
    segment(n) = 8t + p//16 - all 4 of a partition's nodes share a segment.

    feat is cast fp32->bf16 in flight by a gpsimd(SWDGE) DMA. Score pass:
    one bf16 tensor_mul (2x DVE mode) + bf16 halving tree-folds + a small
    fp32 tail reduce. Segment softmax denominators: per-partition j-reduce,
    then two tiny fixed-mask PE matmuls (16-partition group sum + broadcast
    back). Weighted sums: per-(t,j) matmuls with [P,8] alpha stationaries
    accumulating over j; psum partition row == segment, so the output DMA
    is a single clean pattern. No transposes, no 1/w fixup.
    """
    import concourse.bacc as bacc
    import concourse.tile as tile
    from concourse import mybir

    TC, J = 8, 4
    MACRO_NODES = P * TC * J          # 4096
    MACRO_SEGS = 2 * T                # 64
    MACROS = n_macros or (NODES_C // MACRO_NODES)   # 32
    NODES = MACROS * MACRO_NODES
    SEGS = MACROS * MACRO_SEGS
    f32 = mybir.dt.float32
    bf = mybir.dt.bfloat16
    Alu = mybir.AluOpType
    Act = mybir.ActivationFunctionType
    AxX = mybir.AxisListType.X

    nc = bacc.Bacc("TRN2", target_bir_lowering=False, debug=False)
    feat = nc.dram_tensor("feat", [NODES, D], f32, kind="ExternalInput")
    wbb_d = nc.dram_tensor("wbb", [P, D], bf, kind="ExternalInput")
    mask8b_d = nc.dram_tensor("mask8b", [P, 32], bf, kind="ExternalInput")
    m16_d = nc.dram_tensor("m16", [P, 8], f32, kind="ExternalInput")
    e16_d = nc.dram_tensor("e16", [8, P], f32, kind="ExternalInput")
    out_d = nc.dram_tensor("out", [SEGS, D], f32, kind="ExternalOutput")
    if dbg:
        dbg_ft = nc.dram_tensor("dbg_ft", [P, TC, J, D], bf, kind="ExternalOutput")
        dbg_scl = nc.dram_tensor("dbg_scl", [P, TC, J], f32, kind="ExternalOutput")
        dbg_alpha = nc.dram_tensor("dbg_alpha", [P, TC, J], f32, kind="ExternalOutput")

    with tile.TileContext(nc) as tc:
        with (
            tc.tile_pool(name="singles", bufs=1) as singles,
            tc.tile_pool(name="featp", bufs=featp_bufs) as featp,
            tc.tile_pool(name="tmpp", bufs=tmpp_bufs) as tmpp,
            tc.tile_pool(name="scorep", bufs=4) as scorep,
            tc.tile_pool(name="small", bufs=4) as small,
            tc.tile_pool(name="a8p", bufs=3) as a8p,
            tc.tile_pool(name="outp", bufs=3) as outp,
            tc.tile_pool(name="ps_a", bufs=2, space="PSUM") as ps_a,
            tc.tile_pool(name="ps_b", bufs=2, space="PSUM") as ps_b,
        ):
            wbb = singles.tile([P, D], bf)
            nc.sync.dma_start(out=wbb[:], in_=wbb_d[:, :])
            mask8b = singles.tile([P, 32], bf)
            nc.sync.dma_start(out=mask8b[:], in_=mask8b_d[:, :])
            m16 = singles.tile([P, 8], f32)
            nc.sync.dma_start(out=m16[:], in_=m16_d[:, :])
            e16 = singles.tile([8, P], f32)
            nc.sync.dma_start(out=e16[:], in_=e16_d[:, :])
            wbb_full = None
            if mat_w:
                wbb_full = singles.tile([P, TC, J, D], bf)
                nc.vector.tensor_copy(
                    wbb_full[:],
                    wbb[:][:, None, None, :].broadcast_to([P, TC, J, D]),
                )

            state = {}

            def phase1(m):
                nb = m * MACRO_NODES
                ft = featp.tile([P, TC, J, D], bf)
                if not skip_load:
                    nc.gpsimd.dma_start(
                        out=ft[:],
                        in_=feat[nb:nb + MACRO_NODES, :].rearrange(
                            "(t p j) d -> p t j d", p=P, j=J
                        ),
                    )
                else:
                    # token write so the tile allocator sees a producer
                    nc.vector.memset(ft[:, 0, 0, :], 0.0)
                tmp = tmpp.tile([P, TC, J, D], bf)
                nc.vector.tensor_mul(
                    tmp[:],
                    ft[:],
                    wbb_full[:] if mat_w else
                    wbb[:][:, None, None, :].broadcast_to([P, TC, J, D]),
                )
                score = scorep.tile([P, TC, J], f32, tag="score")
                if short_score:
                    nc.vector.reduce_sum(
                        score[:], tmp[:, :, :, :16], axis=AxX
                    )
                else:
                    feng = nc.gpsimd if fold_pool else nc.vector
                    cur = tmp
                    w_cur = D
                    while w_cur > fold_stop:
                        nxt = tmpp.tile(
                            [P, TC, J, w_cur // 2], bf,
                            tag=f"s{w_cur // 2}",
                        )
                        eng = nc.vector if w_cur == D else feng
                        eng.tensor_add(
                            nxt[:],
                            cur[:, :, :, :w_cur // 2],
                            cur[:, :, :, w_cur // 2:],
                        )
                        cur = nxt
                        w_cur //= 2
                    nc.vector.reduce_sum(score[:], cur[:], axis=AxX)
                scl = scorep.tile([P, TC, J], f32, tag="scl")
                nc.vector.scalar_tensor_tensor(
                    scl[:], score[:], NEG_SLOPE, score[:], Alu.mult, Alu.max
                )
                if dbg and m == 0:
                    nc.sync.dma_start(out=dbg_ft[:, :, :, :], in_=ft[:])
                    nc.sync.dma_start(out=dbg_scl[:, :, :], in_=scl[:])
                state[m] = (ft, scl)

            def phase2a(m):
                ft, scl = state.pop(m)
                e = small.tile([P, TC, J], bf, tag="e")
                nc.scalar.activation(e[:], scl[:], Act.Exp)
                r = small.tile([P, TC], f32, tag="r")
                nc.vector.reduce_sum(r[:], e[:], axis=AxX)
                d8_ps = ps_a.tile([8, TC], f32, tag="d8")
                nc.tensor.matmul(
                    d8_ps[:], m16[:], r[:], start=True, stop=True
                )
                d8s = small.tile([8, TC], f32, tag="d8s")
                nc.scalar.copy(d8s[:], d8_ps[:])
                dp_ps = ps_a.tile([P, TC], f32, tag="dp")
                nc.tensor.matmul(
                    dp_ps[:], e16[:], d8s[:], start=True, stop=True
                )
                state[("b", m)] = (ft, e, dp_ps)

            def phase2(m):
                ft, e, dp_ps = state.pop(("b", m))
                rden = small.tile([P, TC], f32, tag="rden")
                nc.vector.reciprocal(rden[:], dp_ps[:])
                alpha = small.tile([P, TC, J], bf, tag="alpha")
                nc.vector.tensor_mul(
                    alpha[:],
                    e[:],
                    rden[:][:, :, None].broadcast_to([P, TC, J]),
                )
                # stationary: 8 cols if narrow (psum rows 32k..32k+8 only,
                # copied out in partition-aligned slices), else padded to 32
                SW = 8 if a8_narrow else 32
                A8 = a8p.tile([P, TC, J, SW], bf, tag="A8")
                a8eng = nc.gpsimd if a8_pool else nc.vector
                a8eng.tensor_mul(
                    A8[:],
                    alpha[:][:, :, :, None].broadcast_to([P, TC, J, SW]),
                    mask8b[:, :SW][:, None, None, :].broadcast_to(
                        [P, TC, J, SW]
                    ),
                )
                if dbg and m == 0:
                    af = small.tile([P, TC, J], f32, tag="af")
                    nc.vector.tensor_copy(af[:], alpha[:])
                    nc.sync.dma_start(out=dbg_alpha[:, :, :], in_=af[:])
                osb = outp.tile([P, 2, D], f32, tag="osb")
                ov = out_d[
                    m * MACRO_SEGS:(m + 1) * MACRO_SEGS, :
                ].rearrange("(q k s) d -> k s q d", q=2, k=4, s=8)
                if skip_mm:
                    nc.vector.memset(osb[:], 0.0)
                    for k in range(4):
                        nc.sync.dma_start(
                            out=ov[k], in_=osb[32 * k:32 * k + 8, :, :]
                        )
                    return
                psum_o = ps_b.tile([P, 2, D], f32, tag="po")
                for t in range(TC):
                    k, q = t % 4, t // 4
                    for j in range(J):
                        nc.tensor.matmul(
                            psum_o[32 * k:32 * k + SW, q, :],
                            A8[:, t, j, :],
                            ft[:, t, j, :],
                            start=(j == 0),
                            stop=(j == J - 1),
                            tile_position=(0, 32 * k),
                        )
                # psum row 32k+s, col q  ->  out row 32q + 8k + s
                if psum_dma:
                    for k in range(4):
                        nc.sync.dma_start(
                            out=ov[k], in_=psum_o[32 * k:32 * k + 8, :, :]
                        )
                else:
                    for k in range(4):
                        nc.scalar.copy(
                            osb[32 * k:32 * k + 8, :, :],
                            psum_o[32 * k:32 * k + 8, :, :],
                        )
                        nc.sync.dma_start(
                            out=ov[k], in_=osb[32 * k:32 * k + 8, :, :]
                        )

            for _ in range(repeat):
                for k in range(MACROS + lag):
                    if k >= lag:
                        phase2a(k - lag)
                    if k < MACROS:
                        phase1(k)
                    if k >= lag:
                        phase2(k - lag)
    nc.finalize()
    return nc


def _build_program_ttr(T=T, featp_bufs=5, tmpp_bufs=3, skip_mm=False,
                       skip_load=False, act_lrelu=False, fix_eng="vector",
                       repeat=1, score_mode="tt", pool_every=3):
    """Fused-score variant.

    phase1: one DMA load of the fp32 macro-tile; 32 DVE tensor_tensor_reduce
    ops compute score[p, t] = sum_d ft[p,t,d]*w[d] AND write the product
    tmp = ft*w as bf16 in the same pass (one read of feat instead of two).
    phase2: softmax machinery as before, but the weighted matmuls consume
    the bf16 tmp (2x PE stream rate); the psum->sbuf copy becomes a
    tensor_mul by 1/w[d] (undoes the w factor baked into tmp).
    """
    import concourse.bacc as bacc
    import concourse.tile as tile
    from concourse import mybir

    MACRO_NODES = P * T
    MACRO_SEGS = 2 * T
    MACROS = NODES_C // MACRO_NODES
    f32 = mybir.dt.float32
    bf = mybir.dt.bfloat16
    Alu = mybir.AluOpType
    Act = mybir.ActivationFunctionType
    AxX = mybir.AxisListType.X

    nc = bacc.Bacc("TRN2", target_bir_lowering=False, debug=False)
    feat = nc.dram_tensor("feat", [NODES_C, D], f32, kind="ExternalInput")
    wb_d = nc.dram_tensor("wb", [P, D], f32, kind="ExternalInput")
    rwb_d = nc.dram_tensor("rwb", [P, D], f32, kind="ExternalInput")
    mask2_d = nc.dram_tensor("mask2", [P, 2], f32, kind="ExternalInput")
    mask32_d = nc.dram_tensor(
        "mask32", [P, T // 2, 32], f32, kind="ExternalInput"
    )
    ident_d = nc.dram_tensor("ident", [P, P], f32, kind="ExternalInput")
    out_d = nc.dram_tensor("out", [SEGS_C, D], f32, kind="ExternalOutput")
    TG = T // 2  # tiles per colpack32 accumulation group

    with tile.TileContext(nc) as tc:
        with (
            tc.tile_pool(name="singles", bufs=1) as singles,
            tc.tile_pool(name="featp", bufs=featp_bufs) as featp,
            tc.tile_pool(name="tmpp", bufs=tmpp_bufs) as tmpp,
            tc.tile_pool(name="scorep", bufs=4) as scorep,
            tc.tile_pool(name="small", bufs=4) as small,
            tc.tile_pool(name="ps_a", bufs=2, space="PSUM") as ps_a,
            tc.tile_pool(name="ps_b", bufs=2, space="PSUM") as ps_b,
            tc.tile_pool(name="outp2", bufs=3) as outp2,
            tc.tile_pool(name="a32p", bufs=2) as a32p,
        ):
            wb = singles.tile([P, D], f32)
            nc.sync.dma_start(out=wb[:], in_=wb_d[:, :])
            rwb = singles.tile([P, D], f32)
            nc.sync.dma_start(out=rwb[:], in_=rwb_d[:, :])
            mask2 = singles.tile([P, 2], f32)
            nc.sync.dma_start(out=mask2[:], in_=mask2_d[:, :])
            mask32 = singles.tile([P, TG, 32], f32)
            nc.sync.dma_start(out=mask32[:], in_=mask32_d[:, :, :])
            ident = singles.tile([P, P], f32)
            nc.sync.dma_start(out=ident[:], in_=ident_d[:, :])
            wb_full = None
            if pool_every or score_mode == "tt":
                # materialized broadcast of w for engines that can't take
                # broadcast access patterns (gpsimd)
                wb_full = singles.tile([P, T, D], f32)
                nc.vector.tensor_copy(
                    wb_full[:], wb[:][:, None, :].broadcast_to([P, T, D])
                )

            state = {}

            def phase1(m):
                nb = m * MACRO_NODES
                ft = featp.tile([P, T, D], f32)
                if not skip_load:
                    nc.sync.dma_start(
                        out=ft[:],
                        in_=feat[nb:nb + MACRO_NODES, :].rearrange(
                            "(t p) d -> p t d", p=P
                        ),
                    )
                tmp = tmpp.tile([P, T, D], bf)
                score = scorep.tile([P, T], f32, tag="score")
                use_pool = pool_every and (m % pool_every == 0)
                if use_pool:
                    nc.gpsimd.tensor_mul(tmp[:], ft[:], wb_full[:])
                    nc.vector.reduce_sum(score[:], tmp[:], axis=AxX)
                elif score_mode == "tt":
                    nc.vector.tensor_mul(tmp[:], ft[:], wb_full[:])
                    nc.vector.reduce_sum(score[:], tmp[:], axis=AxX)
                else:
                    for t in range(T):
                        nc.vector.tensor_tensor_reduce(
                            out=tmp[:, t, :],
                            in0=ft[:, t, :],
                            in1=wb[:],
                            scale=1.0,
                            scalar=0.0,
                            op0=Alu.mult,
                            op1=Alu.add,
                            accum_out=score[:, t:t + 1],
                        )
                scl = scorep.tile([P, T], f32, tag="scl")
                if act_lrelu:
                    nc.scalar.activation(
                        scl[:], score[:], Act.Lrelu, alpha=NEG_SLOPE
                    )
                else:
                    nc.vector.scalar_tensor_tensor(
                        scl[:], score[:], NEG_SLOPE, score[:], Alu.mult, Alu.max
                    )
                state[m] = (tmp, scl)

            def phase2(m):
                tmp, scl = state.pop(m)
                sT_ps = ps_a.tile([T, P], f32, tag="sT")
                nc.tensor.transpose(sT_ps[:], scl[:], ident[:])
                # exp straight from PSUM (no max-shift; scores are O(3))
                e = small.tile([T, P], f32, tag="e")
                nc.scalar.activation(e[:], sT_ps[:], Act.Exp)
                e3 = e[:].rearrange("t (g s) -> t g s", g=2)
                den = small.tile([T, 2], f32, tag="den")
                nc.vector.reduce_sum(den[:], e3, axis=AxX)
                rden = small.tile([T, 2], f32, tag="rden")
                nc.vector.reciprocal(rden[:], den[:])
                alpha = small.tile([T, P], f32, tag="alpha")
                nc.scalar.mul(alpha[:, :S], e[:, :S], rden[:, 0:1])
                nc.scalar.mul(alpha[:, S:], e[:, S:], rden[:, 1:2])
                aT_ps = ps_a.tile([P, T], f32, tag="aT")
                nc.tensor.transpose(aT_ps[:], alpha[:], ident[:T, :T])
                acol = small.tile([P, T], f32, tag="acol")
                nc.scalar.copy(acol[:], aT_ps[:])
                if skip_mm:
                    osb = outp2.tile([MACRO_SEGS, D], f32, tag="osb")
                    nc.vector.tensor_copy(
                        osb[:], acol[:MACRO_SEGS, 0:1].broadcast_to(
                            [MACRO_SEGS, D]
                        )
                    )
                    nc.sync.dma_start(
                        out=out_d[m * MACRO_SEGS:(m + 1) * MACRO_SEGS, :],
                        in_=osb[:],
                    )
                    return
                # colpack32: group g = tiles [g*TG, (g+1)*TG); stationary
                # A32[p, s] = alpha[p, t] * mask32[p, u, s] (s = 2u + p//64,
                # u = t - g*TG); 16 matmuls accumulate psum rows 32g..32g+31
                # so psum partition row == segment index within the macro.
                psum_o = ps_b.tile([MACRO_SEGS, D], f32, tag="po")
                A32 = a32p.tile([P, T, 32], bf, tag="A32")
                for g in (0, 1):
                    nc.vector.tensor_mul(
                        A32[:, g * TG:(g + 1) * TG, :],
                        mask32[:],
                        acol[:, g * TG:(g + 1) * TG][:, :, None].broadcast_to(
                            [P, TG, 32]
                        ),
                    )
                    for u in range(TG):
                        nc.tensor.matmul(
                            psum_o[32 * g:32 * (g + 1), :],
                            A32[:, g * TG + u, :],
                            tmp[:, g * TG + u, :],
                            start=(u == 0),
                            stop=(u == TG - 1),
                            tile_position=(0, 32 * g),
                        )
                osb = outp2.tile([MACRO_SEGS, D], f32, tag="osb2")
                # osb = psum * (1/w[d]): undo the w factor baked into tmp
                nc.vector.tensor_mul(osb[:], psum_o[:], rwb[:MACRO_SEGS, :])
                nc.sync.dma_start(
                    out=out_d[m * MACRO_SEGS:(m + 1) * MACRO_SEGS, :],
                    in_=osb[:],
                )

            for _ in range(repeat):
                for k in range(MACROS + 1):
                    if k < MACROS:
                        phase1(k)
                    if k >= 1:
                        phase2(k - 1)
    nc.finalize()
    return nc


def _build_program(T=T, gpsimd_every=0, featp_bufs=6, tmpp_bufs=3, skip_mm=False, skip_out=False, colpack=True, bf16=False, mat_wb=False, act_lrelu=False, act_alpha=True, dve_mat=True, repeat=1):
    """gpsimd_every: macros with m % gpsimd_every == 0 run the score multiply
    on DVE; the rest on GPSIMD. 0 = all on DVE."""
    import concourse.bacc as bacc
    import concourse.tile as tile
    from concourse import mybir

    MACRO_NODES = P * T
    MACRO_SEGS = 2 * T
    MACROS = NODES_C // MACRO_NODES
    f32 = mybir.dt.float32
    bf = mybir.dt.bfloat16
    fdt = bf if bf16 else f32
    Alu = mybir.AluOpType
    Act = mybir.ActivationFunctionType
    AxX = mybir.AxisListType.X

    nc = bacc.Bacc("TRN2", target_bir_lowering=False, debug=False)
    feat = nc.dram_tensor("feat", [NODES_C, D], f32, kind="ExternalInput")
    wb_d = nc.dram_tensor("wb", [P, D], fdt, kind="ExternalInput")
    mask2_d = nc.dram_tensor("mask2", [P, 2], f32, kind="ExternalInput")
    ident_d = nc.dram_tensor("ident", [P, P], f32, kind="ExternalInput")
    out_d = nc.dram_tensor("out", [SEGS_C, D], f32, kind="ExternalOutput")

    with tile.TileContext(nc) as tc:
        with (
            tc.tile_pool(name="singles", bufs=1) as singles,
            tc.tile_pool(name="featp", bufs=featp_bufs) as featp,
            tc.tile_pool(name="tmpp", bufs=tmpp_bufs) as tmpp,
            tc.tile_pool(name="scorep", bufs=4) as scorep,
            tc.tile_pool(name="small", bufs=4) as small,
            tc.tile_pool(name="outp", bufs=3) as outp,
            tc.tile_pool(name="ps_a", bufs=2, space="PSUM") as ps_a,
            tc.tile_pool(name="ps_b", bufs=2, space="PSUM") as ps_b,
            tc.tile_pool(name="outp2", bufs=3) as outp2,
        ):
            wb = singles.tile([P, D], fdt)
            nc.sync.dma_start(out=wb[:], in_=wb_d[:, :])
            mask2 = singles.tile([P, 2], f32)
            nc.sync.dma_start(out=mask2[:], in_=mask2_d[:, :])
            ident = singles.tile([P, P], f32)
            nc.sync.dma_start(out=ident[:], in_=ident_d[:, :])
            wb_full = None
            if mat_wb or dve_mat:
                wb_full = singles.tile([P, T, D], fdt)
                nc.vector.tensor_copy(
                    wb_full[:], wb[:][:, None, :].broadcast_to([P, T, D])
                )

            state = {}

            def phase1(m):
                nb = m * MACRO_NODES
                ft = featp.tile([P, T, D], fdt)
                ld_eng = nc.gpsimd if bf16 else nc.sync
                ld_eng.dma_start(
                    out=ft[:],
                    in_=feat[nb:nb + MACRO_NODES, :].rearrange(
                        "(t p) d -> p t d", p=P
                    ),
                )
                tmp = tmpp.tile([P, T, D], fdt)
                use_gp = gpsimd_every and (m % gpsimd_every != 0)
                meng = nc.gpsimd if use_gp else nc.vector
                wsrc = (
                    wb_full[:]
                    if ((use_gp or dve_mat) and wb_full is not None)
                    else wb[:][:, None, :].broadcast_to([P, T, D])
                )
                meng.tensor_mul(tmp[:], ft[:], wsrc)
                score = scorep.tile([P, T], f32, tag="score")
                nc.vector.reduce_sum(score[:], tmp[:], axis=AxX)
                scl = scorep.tile([P, T], f32, tag="scl")
                if act_lrelu:
                    nc.scalar.activation(
                        scl[:], score[:], Act.Lrelu, alpha=NEG_SLOPE
                    )
                else:
                    nc.vector.scalar_tensor_tensor(
                        scl[:], score[:], NEG_SLOPE, score[:], Alu.mult, Alu.max
                    )
                state[m] = (ft, scl)

            def phase2(m):
                ft, scl = state.pop(m)
                sT_ps = ps_a.tile([T, P], f32, tag="sT")
                nc.tensor.transpose(sT_ps[:], scl[:], ident[:])
                # exp straight from PSUM (no max-shift; scores are O(3))
                e = small.tile([T, P], f32, tag="e")
                nc.scalar.activation(e[:], sT_ps[:], Act.Exp)
                e3 = e[:].rearrange("t (g s) -> t g s", g=2)
                den = small.tile([T, 2], f32, tag="den")
                nc.vector.reduce_sum(den[:], e3, axis=AxX)
                rden = small.tile([T, 2], f32, tag="rden")
                nc.vector.reciprocal(rden[:], den[:])
                alpha = small.tile([T, P], f32, tag="alpha")
                alpha3 = alpha[:].rearrange("t (g s) -> t g s", g=2)
                if act_alpha:
                    nc.scalar.mul(alpha[:, :S], e[:, :S], rden[:, 0:1])
                    nc.scalar.mul(alpha[:, S:], e[:, S:], rden[:, 1:2])
                else:
                    nc.vector.tensor_mul(
                        alpha3, e3, rden[:][:, :, None].broadcast_to([T, 2, S])
                    )
                aT_ps = ps_a.tile([P, T], f32, tag="aT")
                nc.tensor.transpose(aT_ps[:], alpha[:], ident[:T, :T])
                acol = small.tile([P, T], f32, tag="acol")
                nc.scalar.copy(acol[:], aT_ps[:])
                A = small.tile([P, T, 2], fdt, tag="A")
                nc.vector.tensor_mul(
                    A[:],
                    mask2[:][:, None, :].broadcast_to([P, T, 2]),
                    acol[:][:, :, None].broadcast_to([P, T, 2]),
                )
                if colpack:
                    # col-packed: psum_o[32j+b, q, d] = out row of seg 2t+b,
                    # t = 4q+j; stationary = tiny A-pair (LDW P=2), rhs = ft.
                    QG = T // 4
                    psum_o = ps_b.tile([P, QG, D], f32, tag="po")
                    for t in range(T):
                        q, j = divmod(t, 4)
                        nc.tensor.matmul(
                            psum_o[32 * j:32 * j + 2, q, :],
                            A[:, t, :],
                            ft[:, t, :],
                            start=True,
                            stop=True,
                            tile_position=(0, 32 * j),
                        )
                    osb = outp2.tile([P, QG, D], f32, tag="osb2")
                    nc.scalar.copy(osb[:], psum_o[:])
                    for b in (0, 1):
                        nc.sync.dma_start(
                            out=out_d[
                                m * MACRO_SEGS + b:(m + 1) * MACRO_SEGS:2, :
                            ].rearrange("(q j) d -> j q d", j=4),
                            in_=osb[b::32, :, :],
                        )
                    return
                # oT[d, 2t+b] = sum_p ft[p, t, d] * A[p, t, b]
                if skip_mm:
                    osb = outp.tile([MACRO_SEGS, P], f32, tag="osb")
                    Af = A[:].rearrange("p t b -> p (t b)")
                    nc.scalar.copy(osb[:, :MACRO_SEGS], Af[:MACRO_SEGS, :MACRO_SEGS])
                    nc.scalar.copy(osb[:, MACRO_SEGS:], Af[MACRO_SEGS:2 * MACRO_SEGS, :P - MACRO_SEGS])
                    nc.sync.dma_start(
                        out=out_d[m * MACRO_SEGS:(m + 1) * MACRO_SEGS, :], in_=osb[:]
                    )
                    return
                oT_ps = ps_b.tile([P, MACRO_SEGS], f32, tag="oT")
                for t in range(T):
                    nc.tensor.matmul(
                        oT_ps[:, 2 * t:2 * t + 2],
                        ft[:, t, :],
                        A[:, t, :],
                        start=True,
                        stop=True,
                    )
                if skip_out:
                    osb = outp.tile([MACRO_SEGS, P], f32, tag="osb")
                    nc.scalar.copy(osb[:, :MACRO_SEGS], oT_ps[:MACRO_SEGS, :])
                    nc.scalar.copy(osb[:, MACRO_SEGS:], oT_ps[MACRO_SEGS:2 * MACRO_SEGS, :P - MACRO_SEGS])
                    nc.sync.dma_start(
                        out=out_d[m * MACRO_SEGS:(m + 1) * MACRO_SEGS, :], in_=osb[:]
                    )
                    return
                oT = outp.tile([P, MACRO_SEGS], f32, tag="oTs")
                nc.scalar.copy(oT[:], oT_ps[:])
                o_ps = ps_b.tile([MACRO_SEGS, P], f32, tag="o")
                nc.tensor.transpose(o_ps[:], oT[:], ident[:])
                osb = outp.tile([MACRO_SEGS, P], f32, tag="osb")
                nc.scalar.copy(osb[:], o_ps[:])
                nc.sync.dma_start(
                    out=out_d[m * MACRO_SEGS:(m + 1) * MACRO_SEGS, :], in_=osb[:]
                )

            for _ in range(repeat):
                for k in range(MACROS + 1):
                    if k < MACROS:
                        phase1(k)
                    if k >= 1:
                        phase2(k - 1)
    nc.finalize()
    return nc


def kernel(feat, sizes, w):
    global _PROGRAM, LAST_RESULT
    feat = np.ascontiguousarray(np.asarray(feat), dtype=np.float32)
    sizes = np.asarray(sizes)
    w = np.asarray(w, dtype=np.float32).reshape(-1)
    if (
        feat.shape != (N_FULL, D)
        or sizes.shape != (B_FULL,)
        or not bool(np.all(sizes == S))
    ):
        return _numpy_fallback(feat, np.asarray(sizes), w.reshape(D, 1))

    from concourse.bass_utils import run_bass_kernel_spmd

    if _PROGRAM is None:
        _PROGRAM = _build_program_c()

    in_maps = [in_map_for_core(feat, w, c) for c in range(NCORES)]
    res = run_bass_kernel_spmd(
        _PROGRAM, in_maps, core_ids=list(range(NCORES)), trace=TRACE
    )
    LAST_RESULT = res
    return np.concatenate([r["out"] for r in res.results], axis=0)


def in_map_for_core(feat, w, c, bf16=False):
    import ml_dtypes
    wdt = ml_dtypes.bfloat16 if bf16 else np.float32
    wb = np.ascontiguousarray(
        np.broadcast_to(np.asarray(w, np.float32).reshape(1, D), (P, D)),
    ).astype(wdt)
    rwb = np.ascontiguousarray(
        np.broadcast_to(
            (1.0 / np.asarray(w, np.float32)).reshape(1, D), (P, D)
        ),
    ).astype(np.float32)
    mask2 = np.zeros((P, 2), np.float32)
    mask2[:S, 0] = 1.0
    mask2[S:, 1] = 1.0
    TG = T // 2
    mask32 = np.zeros((P, TG, 32), np.float32)
    for p in range(P):
        for u in range(TG):
            mask32[p, u, 2 * u + (p // S)] = 1.0
    ident = np.eye(P, dtype=np.float32)
    # layout-C inputs
    wbb = wb.astype(ml_dtypes.bfloat16)
    m16 = np.zeros((P, 8), np.float32)
    m16[np.arange(P), np.arange(P) // 16] = 1.0
    mask8b = np.zeros((P, 32), ml_dtypes.bfloat16)
    mask8b[np.arange(P), np.arange(P) // 16] = 1.0
    e16 = np.ascontiguousarray(m16.T)
    return {
        "feat": feat[c * NODES_C:(c + 1) * NODES_C],
        "wb": wb,
        "rwb": rwb,
        "mask2": mask2,
        "mask32": mask32,
        "ident": ident,
        "wbb": wbb,
        "mask8b": mask8b,
        "m16": m16,
        "e16": e16,
    }



# revision 28
# speedup vs baseline: 7.6730x; 7.6730x over previous
"""Trainium2 Bass kernel for AttentiveReduce (segment-softmax attention
readout).

reference semantics (uniform segments of S=64 nodes):
    score = leakyrelu(feat @ w, 0.2)             # (N,)
    alpha = segment_softmax(score)               # softmax within each segment
    out[g, :] = sum_{n in seg g} alpha[n] * feat[n, :]    # (B, D)

Sharding: 8 cores, core c owns segments [c*2048, (c+1)*2048) == rows
[c*131072, (c+1)*131072) of feat.  One SPMD Bass program; no collectives.

Per-core layout (J=16 node-major):
  node n = base + t*2048 + p*16 + j   (t < TC tiles, p < 128, j < 16)
  so each partition's DMA chunk is 16 rows = 8 KiB contiguous fp32, and
  seg(n) = 32t + p//4 — all 16 of a partition's nodes share a segment,
  each segment spans 4 partitions x 16 nodes.

Pipeline per macro-tile (TC*2048 nodes):
  load:   gpsimd(SWDGE) DMA with in-flight fp32->bf16 cast (8 KiB chunks
          run at ~400+ GB/s read vs ~360 for 2 KiB), issued two macros
          ahead of compute.
  score:  DVE bf16 tensor_mul by w + halving folds to 8 + one reduce;
          leakyrelu via DVE scalar_tensor_tensor (ACT's Lrelu ignores
          alpha on this toolchain).
  softmax: per-tile exp on ACT with accum_out giving the per-partition
          e-sums for free; per-(t) PE matmuls with a group mask put the
          32 segment denominators per tile directly in psum rows 32t+s;
          one reciprocal (DVE).  The division is NOT applied per node —
          it rides the psum->sbuf copy as a per-partition ACT scale at
          the very end (psum row == segment).
  reduce: A32[p, s, t, j] = e[p,t,j] * mask(s == p//4) (DVE, 2x mode via
          a materialized mask so the innermost axis stays step-1), then
          per-(t,j) TensorE matmuls accumulate sum_e_ft into psum row
          32t + p//4 over j.  Output DMA is one contiguous 64 KiB store
          per macro.
"""

import numpy as np

N_FULL = 1048576
B_FULL = 16384
D = 128
P = 128
S = 64                      # nodes per segment (uniform fast path)
NCORES = 8
NODES_C = N_FULL // NCORES  # 131072
SEGS_C = B_FULL // NCORES   # 2048
J = 16                      # nodes per partition per tile
TPN = P * J                 # 2048 nodes per tile = 1 MiB fp32
NT = NODES_C // TPN         # 64 tiles per core
NEG_SLOPE = 0.2

# macro schedule in tiles: small macros at the start (DVE gets work as
# soon as the first loads land) and at the end (short post-load tail).
TC_SCHED = (1, 2, 4, 4, 4, 4, 4, 4, 4, 4, 4, 4, 4, 4, 4, 4, 3, 1, 1)
assert sum(TC_SCHED) == NT

_PROGRAM = None
TRACE = False
LAST_RESULT = None


def _numpy_fallback(feat, sizes, w):
    """General segment sizes (not expected in practice)."""
    sizes = sizes.astype(np.int64)
    seg_ids = np.repeat(np.arange(len(sizes)), sizes)
    score = (feat.astype(np.float32) @ w.astype(np.float32))[:, 0]
    score = np.where(score >= 0, score, np.float32(NEG_SLOPE) * score)
    B = len(sizes)
    segmax = np.full(B, -np.inf, np.float32)
    np.maximum.at(segmax, seg_ids, score)
    e = np.exp(score - segmax[seg_ids])
    den = np.zeros(B, np.float32)
    np.add.at(den, seg_ids, e)
    a = (e / den[seg_ids])[:, None].astype(np.float32)
    out = np.zeros((B, feat.shape[1]), np.float32)
    np.add.at(out, seg_ids, feat * a)
    return out


def _build_program_e(hw_first=False, warmup=True,
                     featp_bufs=7, tmpp_bufs=2, sched=TC_SCHED):
    import concourse.bacc as bacc
    import concourse.tile as tile
    from concourse import mybir

    f32 = mybir.dt.float32
    bf = mybir.dt.bfloat16
    Alu = mybir.AluOpType
    Act = mybir.ActivationFunctionType
    AxX = mybir.AxisListType.X
    TCX = max(sched)

    nc = bacc.Bacc("TRN2", target_bir_lowering=False, debug=False)
    feat = nc.dram_tensor("feat", [NODES_C, D], f32, kind="ExternalInput")
    wbb_d = nc.dram_tensor("wbb", [P, D], bf, kind="ExternalInput")
    m4f_d = nc.dram_tensor("m4f", [P, 32], f32, kind="ExternalInput")
    m4b_d = nc.dram_tensor("m4b", [P, 32], bf, kind="ExternalInput")
    out_d = nc.dram_tensor("out", [SEGS_C, D], f32, kind="ExternalOutput")

    with tile.TileContext(nc) as tc:
        with (
            tc.tile_pool(name="singles", bufs=1) as singles,
            tc.tile_pool(name="featp", bufs=featp_bufs) as featp,
            tc.tile_pool(name="f32p", bufs=1) as f32p,
            tc.tile_pool(name="tmpp", bufs=tmpp_bufs) as tmpp,
            tc.tile_pool(name="scorep", bufs=4) as scorep,
            tc.tile_pool(name="small", bufs=4) as small,
            tc.tile_pool(name="a32p", bufs=3) as a32p,
            tc.tile_pool(name="outp", bufs=3) as outp,
            tc.tile_pool(name="ps_a", bufs=2, space="PSUM") as ps_a,
            tc.tile_pool(name="ps_b", bufs=2, space="PSUM") as ps_b,
        ):
            wbb = singles.tile([P, D], bf)
            nc.sync.dma_start(out=wbb[:], in_=wbb_d[:, :])
            m4f = singles.tile([P, 32], f32)
            nc.sync.dma_start(out=m4f[:], in_=m4f_d[:, :])
            m4b = singles.tile([P, 32], bf)
            nc.sync.dma_start(out=m4b[:], in_=m4b_d[:, :])
            # mask_full[p, s, t, j] = (s == p//4), innermost j step-1 so
            # the A32 build keeps the DVE 2x perf mode.
            mask_full = singles.tile([P, 32, TCX, J], bf)
            nc.vector.tensor_copy(
                mask_full[:],
                m4b[:][:, :, None, None].broadcast_to([P, 32, TCX, J]),
            )
            if warmup:
                # pay the Q7 IRAM first-call costs for the SWDGE dma and
                # the tensor ucode while the HWDGE first macro streams
                wtile = singles.tile([P, 8], bf, tag="warm")
                nc.gpsimd.dma_start(out=wtile[:, 0:2], in_=feat[0:P, 0:2])
                wred = singles.tile([P, 2], bf, tag="warmr")
                nc.gpsimd.tensor_add(
                    wred[:], wtile[:, 0:2], wtile[:, 2:4]
                )

            state = {}

            def load(m, tc_, nb):
                nodes = tc_ * TPN
                src = feat[nb:nb + nodes, :].rearrange(
                    "(t p j) d -> p t j d", p=P, j=J
                )
                ft = featp.tile([P, tc_, J, D], bf, tag="ft")
                if hw_first and m == 0:
                    ftf = f32p.tile([P, tc_, J, D], f32, tag="ftf")
                    nc.sync.dma_start(out=ftf[:], in_=src)
                    nc.scalar.copy(ft[:], ftf[:])
                else:
                    nc.gpsimd.dma_start(out=ft[:], in_=src)
                state[("ft", m)] = ft

            def phase1(m, tc_, nb):
                ft = state.pop(("ft", m))
                tmp = tmpp.tile([P, tc_, J, D], bf, tag="tmp")
                nc.vector.tensor_mul(
                    tmp[:],
                    ft[:],
                    wbb[:][:, None, None, :].broadcast_to([P, tc_, J, D]),
                )
                # halving folds (bf16, 2x DVE) down to 16, then one reduce
                cur, w_cur = tmp, D
                while w_cur > 8:
                    h = w_cur // 2
                    nxt = tmpp.tile([P, tc_, J, h], bf, tag=f"t{h}")
                    nc.vector.tensor_add(
                        nxt[:], cur[:, :, :, :h], cur[:, :, :, h:]
                    )
                    cur, w_cur = nxt, h
                score = scorep.tile([P, tc_, J], f32, tag="score")
                nc.vector.tensor_reduce(
                    score[:], cur[:], axis=AxX, op=Alu.add
                )
                scl = scorep.tile([P, tc_, J], f32, tag="scl")
                nc.vector.scalar_tensor_tensor(
                    scl[:], score[:], NEG_SLOPE, score[:], Alu.mult, Alu.max
                )
                state[m] = (ft, scl)

            def phase2(m, tc_, sb):
                ft, scl = state.pop(m)
                e = small.tile([P, tc_, J], bf, tag="e")
                r = small.tile([P, tc_], f32, tag="r")
                for t in range(tc_):
                    nc.scalar.activation(
                        e[:, t, :], scl[:, t, :], Act.Exp,
                        accum_out=r[:, t:t + 1],
                    )
                # segment denominators straight into psum rows 32t+s
                dcol_ps = ps_a.tile([P, 1], f32, tag="dcol")
                for t in range(tc_):
                    nc.tensor.matmul(
                        dcol_ps[32 * t:32 * t + 32, :],
                        m4f[:],
                        r[:, t:t + 1],
                        start=True,
                        stop=True,
                        tile_position=(0, 32 * t),
                    )
                rden = small.tile([P, 1], f32, tag="rden")
                nc.vector.reciprocal(
                    rden[:32 * tc_, :], dcol_ps[:32 * tc_, :]
                )
                A32 = a32p.tile([P, 32, tc_, J], bf, tag="A32")
                nc.vector.tensor_mul(
                    A32[:],
                    mask_full[:, :, :tc_, :],
                    e[:][:, None, :, :].broadcast_to([P, 32, tc_, J]),
                )
                psum_o = ps_b.tile([P, D], f32, tag="po")
                for t in range(tc_):
                    for j in range(J):
                        nc.tensor.matmul(
                            psum_o[32 * t:32 * t + 32, :],
                            A32[:, :, t, j],
                            ft[:, t, j, :],
                            start=(j == 0),
                            stop=(j == J - 1),
                            tile_position=(0, 32 * t),
                        )
                osb = outp.tile([P, D], f32, tag="osb")
                # normalization rides the psum->sbuf copy: row 32t+s of
                # psum_o is segment sb+32t+s, scaled by its 1/denominator
                nc.scalar.activation(
                    osb[:32 * tc_, :],
                    psum_o[:32 * tc_, :],
                    Act.Copy,
                    scale=rden[:32 * tc_, :],
                )
                nc.sync.dma_start(
                    out=out_d[sb:sb + 32 * tc_, :], in_=osb[:32 * tc_, :]
                )

            nb = 0
            info = []
            for m, tc_ in enumerate(sched):
                info.append((m, tc_, nb, nb // S))
                nb += tc_ * TPN
            LA = 2  # loads run this many macros ahead of compute
            n = len(sched)
            for k in range(n + LA + 1):
                if k < n:
                    m, tc_, nbk, _ = info[k]
                    load(m, tc_, nbk)
                if LA <= k < n + LA:
                    m, tc_, nbk, _ = info[k - LA]
                    phase1(m, tc_, nbk)
                if k > LA:
                    m, tc_, _, sbk = info[k - LA - 1]
                    phase2(m, tc_, sbk)
    nc.finalize()
    return nc


def _in_map_for_core(feat, w, c):
    import ml_dtypes

    wb = np.ascontiguousarray(
        np.broadcast_to(np.asarray(w, np.float32).reshape(1, D), (P, D))
    )
    m4 = np.zeros((P, 32), np.float32)
    m4[np.arange(P), np.arange(P) // 4] = 1.0
    return {
        "feat": feat[c * NODES_C:(c + 1) * NODES_C],
        "wbb": wb.astype(ml_dtypes.bfloat16),
        "m4f": m4,
        "m4b": m4.astype(ml_dtypes.bfloat16),
    }


def kernel(feat, sizes, w):
    global _PROGRAM, LAST_RESULT
    feat = np.ascontiguousarray(np.asarray(feat), dtype=np.float32)
    sizes = np.asarray(sizes)
    w = np.asarray(w, dtype=np.float32).reshape(-1)
    if (
        feat.shape != (N_FULL, D)
        or sizes.shape != (B_FULL,)
        or not bool(np.all(sizes == S))
    ):
        return _numpy_fallback(feat, np.asarray(sizes), w.reshape(D, 1))

    from concourse.bass_utils import run_bass_kernel_spmd

    if _PROGRAM is None:
        _PROGRAM = _build_program_e()

    in_maps = [_in_map_for_core(feat, w, c) for c in range(NCORES)]
    res = run_bass_kernel_spmd(
        _PROGRAM, in_maps, core_ids=list(range(NCORES)), trace=TRACE
    )
    LAST_RESULT = res
    return np.concatenate([r["out"] for r in res.results], axis=0)
